# revision 19
# baseline (speedup 1.0000x reference)
"""Causal self-attention (B=2, T=4096, C=768, H=12, D=64, RoPE) on 8 TRN2 cores.

Sharding: core c handles batch b = c//4 and heads [3g, 3g+1, 3g+2] with g = c%4
(data parallel over B, tensor parallel over heads). Each core computes its
heads' QKV projections, RoPE, causal attention and the partial output
projection; the host sums the 4 partial projections per batch.

All device activations/weights are bf16 (pre-cast on the host); PSUM stays
fp32.  The kernel runs a single wavefront over 512-column superchunks so the
tensor engine is continuously fed (QKV projection, attention and output
projection for successive superchunks interleave):

  for s: B(s) = QKV+rope+v^T for cols [512s,512s+512)
         D(s-1) = output projection for the previous superchunk
         C(s)  = attention for query superchunk s, all 3 heads

Device-side layout (per core):
  - x ships transposed bf16: xT [768, 4096].
  - q/k are channel-major in three 128-row m-chunks (q01 | k01 | q2+k2),
    each head's channels permuted [even|odd] so RoPE is two elementwise
    muls, a 32-partition swap (SBUF->SBUF DMA) and an add.
  - v^T is computed directly (x^T chunks stationary, Wv moving) into
    v_aug key-major tiles with a ones column per head (65-wide slots) so
    PV accumulates the softmax denominator for free.
  - S^T = k-blocks^T q lands in PSUM; exp runs on the scalar engine
    (PSUM -> SBUF bf16); diagonal blocks get a bf16 mask multiply (DVE).
  - PV is transposed: P^T 128-query chunks are stationary, v_aug moving;
    y arrives [queries, channels] with a per-query denominator, normalized
    by gpsimd.normalize_recip and transposed back to channel-major on the
    tensor engine.
  - the output projection emits outT [768, 4096] fp32 partials.
"""

import sys

sys.path.insert(0, "/opt/trn_rl_repo")

from contextlib import ExitStack

import numpy as np
import ml_dtypes

import concourse.bass as bass
import concourse.tile as tile
from concourse import bacc, mybir
from concourse.bass_utils import run_bass_kernel_spmd
from concourse.masks import make_identity

P = 128
C = 768
D = 64
HPC = 3            # heads per core
DQ = HPC * D       # 192 channels per core
W3 = 3 * DQ        # 576 = q+k+v output channels per core
KCH = C // P       # 6 contraction chunks for projections
SCQ = 512          # superchunk (phase B cols == query block == proj cols)
GK = 2             # S^T tiles per exp group
VW = HPC * 65      # v_aug row width per key chunk (3 heads x (64 + ones))

f32 = mybir.dt.float32
bf16 = mybir.dt.bfloat16
EXP = mybir.ActivationFunctionType.Exp
BF = ml_dtypes.bfloat16


def build(T=4096, n_cores=8):
    NSC = T // SCQ
    nc = bacc.Bacc("TRN2", target_bir_lowering=False, debug=False,
                   num_devices=n_cores)

    xT_d = nc.dram_tensor("xT", [C, T], bf16, kind="ExternalInput").ap()
    w_d = nc.dram_tensor("w", [C, W3], bf16, kind="ExternalInput").ap()
    wp_d = nc.dram_tensor("wp", [DQ, C], bf16, kind="ExternalInput").ap()
    cp_d = nc.dram_tensor("cpat", [P, T], bf16, kind="ExternalInput").ap()
    sp_d = nc.dram_tensor("spat", [P, T], bf16, kind="ExternalInput").ap()
    mk_d = [nc.dram_tensor(f"mk{d}", [P, SCQ], bf16, kind="ExternalInput").ap()
            for d in range(4)]
    out_d = nc.dram_tensor("outT", [C, T], f32, kind="ExternalOutput").ap()

    xT_v = xT_d.rearrange("(a p) t -> p a t", p=P)
    w_v = w_d.rearrange("(a p) n -> p a n", p=P)

    with tile.TileContext(nc) as tc, ExitStack() as top:
        const = top.enter_context(tc.tile_pool(name="const", bufs=1))
        persist = top.enter_context(tc.tile_pool(name="persist", bufs=1))
        xp = top.enter_context(tc.tile_pool(name="xp", bufs=2))
        rtmp = top.enter_context(tc.tile_pool(name="rtmp", bufs=2))
        ptp = top.enter_context(tc.tile_pool(name="ptp", bufs=3))
        smp = top.enter_context(tc.tile_pool(name="smp", bufs=2))
        otp = top.enter_context(tc.tile_pool(name="otp", bufs=3))
        # PSUM: shared [128,512] ring (QKV / v^T / proj) + attention pools
        g_ps = top.enter_context(tc.tile_pool(name="g_ps", bufs=2,
                                              space="PSUM"))
        s_ps = top.enter_context(tc.tile_pool(name="s_ps", bufs=2,
                                              space="PSUM"))
        y_ps = top.enter_context(tc.tile_pool(name="y_ps", bufs=1,
                                              space="PSUM"))
        tr_ps = top.enter_context(tc.tile_pool(name="tr_ps", bufs=1,
                                               space="PSUM"))

        # --- constants ---
        w_r = const.tile([P, KCH, W3], bf16)
        wpA = const.tile([P, C], bf16)
        wpB = const.tile([DQ - P, C], bf16)
        cp_sb = const.tile([P, T], bf16)
        sp_sb = const.tile([P, T], bf16)
        mk_sb = [const.tile([P, SCQ], bf16, tag=f"mk{d}", name=f"mk{d}")
                 for d in range(4)]
        ident = const.tile([P, P], bf16)

        # persistent activations
        qt1 = persist.tile([P, T], bf16)
        kt1 = persist.tile([P, T], bf16)
        qt2 = persist.tile([D, T], bf16)
        kt2 = persist.tile([D, T], bf16)
        v_aug = persist.tile([P, (T // P) * VW], bf16)
        yt1 = persist.tile([P, T], bf16)
        yt2 = persist.tile([D, T], bf16)
        v4 = v_aug[:].rearrange("p (a h c) -> p a h c", h=HPC, c=65)

        # preamble: xb(0) first on sync so the PE can start ASAP; the
        # remaining constants go behind it / on the scalar queue.
        xb0 = xp.tile([P, KCH, SCQ], bf16, tag="xb")
        nc.sync.dma_start(xb0[:], xT_v[:, :, 0:SCQ])
        nc.scalar.dma_start(w_r[:], w_v[:])
        nc.sync.dma_start(cp_sb[:], cp_d[:])
        nc.scalar.dma_start(sp_sb[:], sp_d[:])
        for d in range(4):
            (nc.sync if d % 2 else nc.scalar).dma_start(mk_sb[d][:],
                                                        mk_d[d][:])
        nc.sync.dma_start(wpA[:], wp_d[0:P, :])
        nc.scalar.dma_start(wpB[:], wp_d[P:DQ, :])
        make_identity(nc, ident[:])
        nc.gpsimd.memset(v4[:, :, :, 64], 1.0)
        # prefetch the exp activation table so the first real exp is cheap
        warm = const.tile([1, 8], f32)
        nc.scalar.activation(warm[:], ident[0:1, 0:8], EXP, scale=1.0)

        st = {"pend_pv": None, "pend_tr": None, "xb": xb0}

        def emit_pv(psy, pend):
            pt, h, s, g0, gsz = pend
            nkj = 4 * s + 4
            for j in range(gsz):
                kj = g0 + j
                va = v_aug[:, kj * VW + h * 65: kj * VW + (h + 1) * 65]
                for qc in range(4):
                    nc.tensor.matmul(
                        psy[:, qc * 65:(qc + 1) * 65],
                        pt[:, j * SCQ + qc * P: j * SCQ + (qc + 1) * P],
                        va,
                        start=(kj == 0 and qc == 0),
                        stop=(kj == nkj - 1 and qc == 3))

        def emit_y_norm(psy, h, s):
            """copy + per-query normalize (DVE + Pool only, no PE)."""
            ysb = smp.tile([P, 4 * 65], f32, tag="ysb")
            nc.vector.tensor_copy(ysb[:], psy[:])
            yn = smp.tile([P, 4 * D], bf16, tag="yn")
            for qc in range(4):
                nc.gpsimd.normalize_recip(
                    yn[:, qc * D:(qc + 1) * D],
                    ysb[:, qc * 65:qc * 65 + D],
                    ysb[:, qc * 65 + D:qc * 65 + D + 1])
            return yn

        def emit_y_tr(pend_tr):
            """transpose y back to channel-major and store (PE + one copy)."""
            yn, h, s = pend_tr
            ytr = tr_ps.tile([D, 4 * P], bf16, tag="tr")
            for qc in range(4):
                nc.tensor.transpose(ytr[:, qc * P:(qc + 1) * P],
                                    yn[:, qc * D:(qc + 1) * D], ident[:])
            scols = slice(s * SCQ, (s + 1) * SCQ)
            ydst = yt1[h * D:(h + 1) * D, scols] if h < 2 else yt2[:, scols]
            nc.vector.tensor_copy(ydst, ytr[:])

        def step_pipeline(new_pend):
            """emit the pending PV; the delayed transpose of the unit before
            last; and, when the pending PV closes a unit, its normalize.
            (every unit has >= 2 groups, so pend_tr always drains before the
            next unit closes)"""
            if st["pend_pv"] is not None:
                ppsy, pend, last = st["pend_pv"]
                emit_pv(ppsy, pend)
                if st["pend_tr"] is not None:
                    emit_y_tr(st["pend_tr"])
                    st["pend_tr"] = None
                if last:
                    yn = emit_y_norm(ppsy, pend[1], pend[2])
                    st["pend_tr"] = (yn, pend[1], pend[2])
            st["pend_pv"] = new_pend

        def b_thunks(s, xb):
            """Superchunk s's QKV+rope+v^T as 7 closures (3 m-chunks + 4
            v^T key-blocks), to be sprinkled between attention groups."""
            cols = slice(s * SCQ, (s + 1) * SCQ)
            bst = {"pend": None}

            def mk_m(mi, dst):
                def f():
                    ps = g_ps.tile([P, SCQ], f32, tag="ps")
                    for kc in range(KCH):
                        nc.tensor.matmul(
                            ps[:], w_r[:, kc, mi * P:(mi + 1) * P],
                            xb[:, kc, :],
                            start=(kc == 0), stop=(kc == KCH - 1))
                    stt = rtmp.tile([P, SCQ], bf16, tag="st")
                    nc.vector.tensor_mul(stt[:], ps[:], sp_sb[:, cols])
                    ct = rtmp.tile([P, SCQ], bf16, tag="ct")
                    nc.vector.tensor_mul(ct[:], ps[:], cp_sb[:, cols])
                    wt = rtmp.tile([P, SCQ], bf16, tag="wt")
                    for p0 in (0, D):
                        nc.sync.dma_start(wt[p0:p0 + 32, :],
                                          stt[p0 + 32:p0 + D, :])
                        nc.sync.dma_start(wt[p0 + 32:p0 + D, :],
                                          stt[p0:p0 + 32, :])
                    if bst["pend"] is not None:
                        _rope_add(nc, bst["pend"], cols)
                    bst["pend"] = (ct, wt, dst)
                return f

            def mk_v(kb):
                def f():
                    psv = g_ps.tile([P, SCQ], f32, tag="ps")
                    for kc in range(KCH):
                        nc.tensor.matmul(
                            psv[:, 0:DQ], xb[:, kc, kb * P:(kb + 1) * P],
                            w_r[:, kc, 2 * DQ:W3],
                            start=(kc == 0), stop=(kc == KCH - 1))
                    if bst["pend"] is not None:
                        _rope_add(nc, bst["pend"], cols)
                        bst["pend"] = None
                    kc32 = s * (SCQ // P) + kb
                    nc.scalar.copy(
                        v4[:, kc32, :, 0:D],
                        psv[:, 0:DQ].rearrange("p (h c) -> p h c", h=HPC))
                return f

            return [mk_m(0, qt1), mk_m(1, kt1), mk_m(2, (qt2, kt2)),
                    mk_v(0), mk_v(1), mk_v(2), mk_v(3)]

        def d_thunks(s):
            """Output projection for superchunk s as 6 closures."""
            cols = slice(s * SCQ, (s + 1) * SCQ)

            def mk(m):
                def f():
                    pso = g_ps.tile([P, SCQ], f32, tag="ps")
                    nc.tensor.matmul(pso[:], wpA[:, m * P:(m + 1) * P],
                                     yt1[:, cols], start=True, stop=False)
                    nc.tensor.matmul(pso[:], wpB[:, m * P:(m + 1) * P],
                                     yt2[:, cols], start=False, stop=True)
                    ot = otp.tile([P, SCQ], f32, tag="ot")
                    nc.vector.tensor_copy(ot[:], pso[:])
                    nc.sync.dma_start(out_d[m * P:(m + 1) * P, cols], ot[:])
                return f

            return [mk(m) for m in range(C // P)]

        # B(0) runs standalone (C(0) needs it immediately)
        for f in b_thunks(0, xb0):
            f()

        for s in range(NSC):
            # filler for this iteration: B(s+1) pieces, then D(s-1) pieces.
            # D pieces may only pop once >= 2 groups have run (the y of
            # (h2, s-1) is emitted during the second group of this C(s)).
            bq, dq = [], []
            if s + 1 < NSC:
                nxb = xp.tile([P, KCH, SCQ], bf16, tag="xb")
                nc.sync.dma_start(nxb[:], xT_v[:, :, (s + 1) * SCQ:
                                                (s + 2) * SCQ])
                bq = b_thunks(s + 1, nxb)
            if s > 0:
                dq = d_thunks(s - 1)
            gcount = 0

            for h in range(HPC):
                if h == 0:
                    q_rows, k_rows = qt1[0:D, :], kt1[0:D, :]
                elif h == 1:
                    q_rows, k_rows = qt1[D:P, :], kt1[D:P, :]
                else:
                    q_rows, k_rows = qt2[:, :], kt2[:, :]
                q_ap = q_rows[:, s * SCQ:(s + 1) * SCQ]
                nkj = 4 * s + 4
                psy = y_ps.tile([P, 4 * 65], f32, tag="y")
                for g0 in range(0, nkj, GK):
                    gsz = min(GK, nkj - g0)
                    pss = s_ps.tile([P, GK * SCQ], f32, tag="ss")
                    for j in range(gsz):
                        kj = g0 + j
                        nc.tensor.matmul(
                            pss[:, j * SCQ:(j + 1) * SCQ],
                            k_rows[:, kj * P:(kj + 1) * P], q_ap,
                            start=True, stop=True)
                    pt = ptp.tile([P, GK * SCQ], bf16, tag="pt")
                    nc.scalar.activation(pt[:, :gsz * SCQ],
                                         pss[:, :gsz * SCQ], EXP,
                                         scale=0.125)
                    for j in range(gsz):
                        kj = g0 + j
                        if kj >= 4 * s:
                            jc = slice(j * SCQ, (j + 1) * SCQ)
                            nc.vector.tensor_mul(pt[:, jc], pt[:, jc],
                                                 mk_sb[kj - 4 * s][:])
                    step_pipeline((psy, (pt, h, s, g0, gsz), g0 + GK >= nkj))
                    gcount += 1
                    if bq:
                        bq.pop(0)()
                    elif dq and gcount >= 2:
                        dq.pop(0)()
            for f in bq + dq:
                f()

        # drain the pipeline
        step_pipeline(None)
        if st["pend_tr"] is not None:
            emit_y_tr(st["pend_tr"])
            st["pend_tr"] = None
        for f in d_thunks(NSC - 1):
            f()

    nc.compile()
    return nc


def _rope_add(nc, pend_rope, cols):
    ct, wt, dst = pend_rope
    if isinstance(dst, tuple):  # (q2, k2) split across two 64-row tiles
        q2, k2 = dst
        nc.vector.tensor_add(q2[:, cols], ct[0:D, :], wt[0:D, :])
        nc.vector.tensor_add(k2[:, cols], ct[D:P, :], wt[D:P, :])
    else:
        nc.vector.tensor_add(dst[:, cols], ct[:], wt[:])


def _emit_proj(nc, s, wpA, wpB, yt1, yt2, g_ps, otp, out_d):
    cols = slice(s * SCQ, (s + 1) * SCQ)
    for m in range(C // P):
        pso = g_ps.tile([P, SCQ], f32, tag="ps")
        nc.tensor.matmul(pso[:], wpA[:, m * P:(m + 1) * P], yt1[:, cols],
                         start=True, stop=False)
        nc.tensor.matmul(pso[:], wpB[:, m * P:(m + 1) * P], yt2[:, cols],
                         start=False, stop=True)
        ot = otp.tile([P, SCQ], f32, tag="ot")
        nc.vector.tensor_copy(ot[:], pso[:])
        nc.sync.dma_start(out_d[m * P:(m + 1) * P, cols], ot[:])


# ---------------------------------------------------------------------------
# host side
# ---------------------------------------------------------------------------

def make_core_inputs(x, Wq, bq, Wk, bk, Wv, bv, Wp, bp, T=4096, n_cores=8):
    """Build the per-core input maps (bf16 device tensors). Biases bq/bk must
    be zero; bv/bp are folded on the host in run()."""
    cpat = np.empty((P, T), dtype=np.float32)
    spat = np.empty((P, T), dtype=np.float32)
    inv_freq = (10000.0 ** (-(np.arange(32, dtype=np.float64)) / 32.0))
    ang = np.arange(T, dtype=np.float64)[None, :] * inv_freq[:, None]  # [32,T]
    cos32 = np.cos(ang).astype(np.float32)
    sin32 = np.sin(ang).astype(np.float32)
    for blk in range(4):
        cpat[blk * 32:(blk + 1) * 32] = cos32
        spat[blk * 32:(blk + 1) * 32] = sin32 if blk % 2 == 0 else -sin32

    jj = np.arange(P)[:, None]
    ii = np.arange(SCQ)[None, :]
    mks = [(jj + P * d <= ii).astype(np.float32) for d in range(4)]

    in_maps = []
    for c in range(n_cores):
        b, g = divmod(c, n_cores // 2)
        heads = [HPC * g + i for i in range(HPC)]

        def eo(h):  # [even d | odd d] rows of head h
            base = D * h
            return [base + 2 * i for i in range(32)] + \
                   [base + 2 * i + 1 for i in range(32)]

        v_rows = []
        for h in heads:
            v_rows += list(range(D * h, D * (h + 1)))
        # m-chunks: [q01 | k01 | q2+k2 | v]
        w_cat = np.concatenate(
            [Wq[eo(heads[0]) + eo(heads[1]), :].T,
             Wk[eo(heads[0]) + eo(heads[1]), :].T,
             Wq[eo(heads[2]), :].T, Wk[eo(heads[2]), :].T,
             Wv[v_rows, :].T],
            axis=1)
        wp_s = np.ascontiguousarray(Wp[:, v_rows].T).astype(BF)
        xT = np.ascontiguousarray(x[b].T).astype(BF)
        im = {
            "xT": xT, "w": np.ascontiguousarray(w_cat).astype(BF),
            "wp": wp_s,
            "cpat": cpat.astype(BF), "spat": spat.astype(BF),
        }
        for d in range(4):
            im[f"mk{d}"] = mks[d].astype(BF)
        in_maps.append(im)
    return in_maps


_nc_cache = {}


def run(x, Wq, bq, Wk, bk, Wv, bv, Wp, bp, T=4096, n_cores=8, trace=False,
        trace_cores=None):
    assert not (np.any(bq) or np.any(bk)), "nonzero q/k bias unsupported"
    key = (T, n_cores)
    if key not in _nc_cache:
        _nc_cache[key] = build(T=T, n_cores=n_cores)
    nc = _nc_cache[key]
    in_maps = make_core_inputs(x, Wq, bq, Wk, bk, Wv, bv, Wp, bp,
                               T=T, n_cores=n_cores)
    res = run_bass_kernel_spmd(nc, in_maps, list(range(n_cores)), trace=trace,
                               trace_cores=trace_cores)
    B = 2
    out = np.zeros((B, T, C), dtype=np.float32)
    for c in range(n_cores):
        b = c // (n_cores // 2)
        out[b] += np.asarray(res.results[c]["outT"], dtype=np.float32).T
    # host-folded bias terms: softmax rows sum to 1, so the v bias passes
    # through attention unchanged: y = att@v + bv  =>  out += bv @ Wp.T + bp
    out += (bv.astype(np.float32) @ Wp.T.astype(np.float32) + bp)[None, None, :]
    return out, res


def kernel(**inputs):
    inputs = {k: np.asarray(v) for k, v in inputs.items()}
    out, _ = run(**inputs)
    return out


# revision 20
# speedup vs baseline: 1.0702x; 1.0702x over previous
"""Causal self-attention (B=2, T=4096, C=768, H=12, D=64, RoPE) on 8 TRN2 cores.

Sharding: core c handles batch b = c//4 and heads [3g, 3g+1, 3g+2] with g = c%4
(data parallel over B, tensor parallel over heads). Each core computes its
heads' QKV projections, RoPE, causal attention and the partial output
projection; the host sums the 4 partial projections per batch.

All device activations/weights are bf16 (pre-cast on the host); PSUM stays
fp32.  The kernel runs a single wavefront over 512-column superchunks so the
tensor engine is continuously fed (QKV projection, attention and output
projection for successive superchunks interleave):

  for s: B(s) = QKV+rope+v^T for cols [512s,512s+512)
         D(s-1) = output projection for the previous superchunk
         C(s)  = attention for query superchunk s, all 3 heads

Device-side layout (per core):
  - x ships transposed bf16: xT [768, 4096].
  - q/k are channel-major in three 128-row m-chunks (q01 | k01 | q2+k2),
    each head's channels permuted [even|odd] so RoPE is two elementwise
    muls, a 32-partition swap (SBUF->SBUF DMA) and an add.
  - v^T is computed directly (x^T chunks stationary, Wv moving) into
    v_aug key-major tiles with a ones column per head (65-wide slots) so
    PV accumulates the softmax denominator for free.
  - S^T = k-blocks^T q lands in PSUM; exp runs on the scalar engine
    (PSUM -> SBUF bf16); diagonal blocks get a bf16 mask multiply (DVE).
  - PV is transposed: P^T 128-query chunks are stationary, v_aug moving;
    y arrives [queries, channels] with a per-query denominator, normalized
    by gpsimd.normalize_recip and transposed back to channel-major on the
    tensor engine.
  - the output projection emits outT [768, 4096] fp32 partials.
"""

import sys

sys.path.insert(0, "/opt/trn_rl_repo")

from contextlib import ExitStack

import numpy as np
import ml_dtypes

import concourse.bass as bass
import concourse.tile as tile
from concourse import bacc, mybir
from concourse.bass_utils import run_bass_kernel_spmd
from concourse.masks import make_identity

P = 128
C = 768
D = 64
HPC = 3            # heads per core
DQ = HPC * D       # 192 channels per core
W3 = 3 * DQ        # 576 = q+k+v output channels per core
KCH = C // P       # 6 contraction chunks for projections
SCQ = 512          # superchunk (phase B cols == query block == proj cols)
GK = 2             # S^T tiles per exp group
VWS = 128          # v_aug slot width (64 v + ones + zero pad = PE filler)
VW = HPC * VWS     # v_aug row width per key chunk

f32 = mybir.dt.float32
bf16 = mybir.dt.bfloat16
EXP = mybir.ActivationFunctionType.Exp
BF = ml_dtypes.bfloat16


def build(T=4096, n_cores=8):
    NSC = T // SCQ
    nc = bacc.Bacc("TRN2", target_bir_lowering=False, debug=False,
                   num_devices=n_cores)

    xT_d = nc.dram_tensor("xT", [C, T], bf16, kind="ExternalInput").ap()
    w_d = nc.dram_tensor("w", [C, W3], bf16, kind="ExternalInput").ap()
    wp_d = nc.dram_tensor("wp", [DQ, C], bf16, kind="ExternalInput").ap()
    cp_d = nc.dram_tensor("cpat", [P, T], bf16, kind="ExternalInput").ap()
    sp_d = nc.dram_tensor("spat", [P, T], bf16, kind="ExternalInput").ap()
    mk_d = [nc.dram_tensor(f"mk{d}", [P, SCQ], bf16, kind="ExternalInput").ap()
            for d in range(4)]
    out_d = nc.dram_tensor("outT", [C, T], f32, kind="ExternalOutput").ap()

    xT_v = xT_d.rearrange("(a p) t -> p a t", p=P)
    w_v = w_d.rearrange("(a p) n -> p a n", p=P)

    with tile.TileContext(nc) as tc, ExitStack() as top:
        const = top.enter_context(tc.tile_pool(name="const", bufs=1))
        persist = top.enter_context(tc.tile_pool(name="persist", bufs=1))
        xp = top.enter_context(tc.tile_pool(name="xp", bufs=2))
        rtmp = top.enter_context(tc.tile_pool(name="rtmp", bufs=2))
        ptp = top.enter_context(tc.tile_pool(name="ptp", bufs=3))
        smp = top.enter_context(tc.tile_pool(name="smp", bufs=2))
        otp = top.enter_context(tc.tile_pool(name="otp", bufs=3))
        # PSUM: shared [128,512] ring (QKV / v^T / proj) + attention pools
        g_ps = top.enter_context(tc.tile_pool(name="g_ps", bufs=2,
                                              space="PSUM"))
        s_ps = top.enter_context(tc.tile_pool(name="s_ps", bufs=2,
                                              space="PSUM"))
        y_ps = top.enter_context(tc.tile_pool(name="y_ps", bufs=1,
                                              space="PSUM"))
        tr_ps = top.enter_context(tc.tile_pool(name="tr_ps", bufs=1,
                                               space="PSUM"))

        # --- constants ---
        w_r = const.tile([P, KCH, W3], bf16)
        wpA = const.tile([P, C], bf16)
        wpB = const.tile([DQ - P, C], bf16)
        cp_sb = const.tile([P, T], bf16)
        sp_sb = const.tile([P, T], bf16)
        mk_sb = [const.tile([P, SCQ], bf16, tag=f"mk{d}", name=f"mk{d}")
                 for d in range(4)]
        ident = const.tile([P, P], bf16)

        # persistent activations
        qt1 = persist.tile([P, T], bf16)
        kt1 = persist.tile([P, T], bf16)
        qt2 = persist.tile([D, T], bf16)
        kt2 = persist.tile([D, T], bf16)
        v_aug = persist.tile([P, (T // P) * VW], bf16)
        yt1 = persist.tile([P, T], bf16)
        yt2 = persist.tile([D, T], bf16)
        v4 = v_aug[:].rearrange("p (a h c) -> p a h c", h=HPC, c=VWS)

        # preamble: xb(0) first on sync so the PE can start ASAP; the
        # remaining constants go behind it / on the scalar queue.
        xb0 = xp.tile([P, KCH, SCQ], bf16, tag="xb")
        nc.sync.dma_start(xb0[:], xT_v[:, :, 0:SCQ])
        nc.scalar.dma_start(w_r[:], w_v[:])
        nc.sync.dma_start(cp_sb[:], cp_d[:])
        nc.scalar.dma_start(sp_sb[:], sp_d[:])
        for d in range(4):
            (nc.sync if d % 2 else nc.scalar).dma_start(mk_sb[d][:],
                                                        mk_d[d][:])
        nc.sync.dma_start(wpA[:], wp_d[0:P, :])
        nc.scalar.dma_start(wpB[:], wp_d[P:DQ, :])
        make_identity(nc, ident[:])
        nc.gpsimd.memset(v_aug[:], 0.0)
        nc.gpsimd.memset(v4[:, :, :, 64], 1.0)
        # prefetch the exp activation table so the first real exp is cheap
        warm = const.tile([1, 8], f32)
        nc.scalar.activation(warm[:], ident[0:1, 0:8], EXP, scale=1.0)

        st = {"pend_pv": None, "pend_tr": None, "xb": xb0}

        def emit_pv(psy, pend):
            pt, h, s, g0, gsz = pend
            nkj = 4 * s + 4
            for j in range(gsz):
                kj = g0 + j
                va = v_aug[:, kj * VW + h * VWS: kj * VW + (h + 1) * VWS]
                for qc in range(4):
                    nc.tensor.matmul(
                        psy[:, qc * VWS:(qc + 1) * VWS],
                        pt[:, j * SCQ + qc * P: j * SCQ + (qc + 1) * P],
                        va,
                        start=(kj == 0 and qc == 0),
                        stop=(kj == nkj - 1 and qc == 3))

        def emit_y_norm(psy, h, s):
            """copy + per-query normalize (DVE + Pool only, no PE)."""
            ysb = smp.tile([P, 4 * VWS], f32, tag="ysb")
            nc.vector.tensor_copy(ysb[:], psy[:])
            yn = smp.tile([P, 4 * D], bf16, tag="yn")
            for qc in range(4):
                nc.gpsimd.normalize_recip(
                    yn[:, qc * D:(qc + 1) * D],
                    ysb[:, qc * VWS:qc * VWS + D],
                    ysb[:, qc * VWS + D:qc * VWS + D + 1])
            return yn

        def emit_y_tr(pend_tr):
            """transpose y back to channel-major and store (PE + one copy)."""
            yn, h, s = pend_tr
            ytr = tr_ps.tile([D, 4 * P], bf16, tag="tr")
            for qc in range(4):
                nc.tensor.transpose(ytr[:, qc * P:(qc + 1) * P],
                                    yn[:, qc * D:(qc + 1) * D], ident[:])
            scols = slice(s * SCQ, (s + 1) * SCQ)
            ydst = yt1[h * D:(h + 1) * D, scols] if h < 2 else yt2[:, scols]
            nc.vector.tensor_copy(ydst, ytr[:])

        def step_pipeline(new_pend):
            """emit the pending PV; the delayed transpose of the unit before
            last; and, when the pending PV closes a unit, its normalize.
            (every unit has >= 2 groups, so pend_tr always drains before the
            next unit closes)"""
            if st["pend_pv"] is not None:
                ppsy, pend, last = st["pend_pv"]
                emit_pv(ppsy, pend)
                if st["pend_tr"] is not None:
                    emit_y_tr(st["pend_tr"])
                    st["pend_tr"] = None
                if last:
                    yn = emit_y_norm(ppsy, pend[1], pend[2])
                    st["pend_tr"] = (yn, pend[1], pend[2])
            st["pend_pv"] = new_pend

        def b_thunks(s, xb):
            """Superchunk s's QKV+rope+v^T as 7 closures (3 m-chunks + 4
            v^T key-blocks), to be sprinkled between attention groups."""
            cols = slice(s * SCQ, (s + 1) * SCQ)
            bst = {"pend": None}

            def mk_m(mi, dst):
                def f():
                    ps = g_ps.tile([P, SCQ], f32, tag="ps")
                    for kc in range(KCH):
                        nc.tensor.matmul(
                            ps[:], w_r[:, kc, mi * P:(mi + 1) * P],
                            xb[:, kc, :],
                            start=(kc == 0), stop=(kc == KCH - 1))
                    stt = rtmp.tile([P, SCQ], bf16, tag="st")
                    nc.vector.tensor_mul(stt[:], ps[:], sp_sb[:, cols])
                    ct = rtmp.tile([P, SCQ], bf16, tag="ct")
                    nc.vector.tensor_mul(ct[:], ps[:], cp_sb[:, cols])
                    wt = rtmp.tile([P, SCQ], bf16, tag="wt")
                    for p0 in (0, D):
                        nc.sync.dma_start(wt[p0:p0 + 32, :],
                                          stt[p0 + 32:p0 + D, :])
                        nc.sync.dma_start(wt[p0 + 32:p0 + D, :],
                                          stt[p0:p0 + 32, :])
                    if bst["pend"] is not None:
                        _rope_add(nc, bst["pend"], cols)
                    bst["pend"] = (ct, wt, dst)
                return f

            def mk_v(kb):
                def f():
                    psv = g_ps.tile([P, SCQ], f32, tag="ps")
                    for kc in range(KCH):
                        nc.tensor.matmul(
                            psv[:, 0:DQ], xb[:, kc, kb * P:(kb + 1) * P],
                            w_r[:, kc, 2 * DQ:W3],
                            start=(kc == 0), stop=(kc == KCH - 1))
                    if bst["pend"] is not None:
                        _rope_add(nc, bst["pend"], cols)
                        bst["pend"] = None
                    kc32 = s * (SCQ // P) + kb
                    nc.vector.tensor_copy(
                        v4[:, kc32, :, 0:D],
                        psv[:, 0:DQ].rearrange("p (h c) -> p h c", h=HPC))
                return f

            return [mk_m(0, qt1), mk_m(1, kt1), mk_m(2, (qt2, kt2)),
                    mk_v(0), mk_v(1), mk_v(2), mk_v(3)]

        def d_thunks(s):
            """Output projection for superchunk s as 6 closures."""
            cols = slice(s * SCQ, (s + 1) * SCQ)

            def mk(m):
                def f():
                    pso = g_ps.tile([P, SCQ], f32, tag="ps")
                    nc.tensor.matmul(pso[:], wpA[:, m * P:(m + 1) * P],
                                     yt1[:, cols], start=True, stop=False)
                    nc.tensor.matmul(pso[:], wpB[:, m * P:(m + 1) * P],
                                     yt2[:, cols], start=False, stop=True)
                    ot = otp.tile([P, SCQ], f32, tag="ot")
                    nc.vector.tensor_copy(ot[:], pso[:])
                    nc.sync.dma_start(out_d[m * P:(m + 1) * P, cols], ot[:])
                return f

            return [mk(m) for m in range(C // P)]

        # B(0) runs standalone (C(0) needs it immediately)
        for f in b_thunks(0, xb0):
            f()

        for s in range(NSC):
            # filler for this iteration: B(s+1) pieces, then D(s-1) pieces.
            # D pieces may only pop once >= 2 groups have run (the y of
            # (h2, s-1) is emitted during the second group of this C(s)).
            bq, dq = [], []
            if s + 1 < NSC:
                nxb = xp.tile([P, KCH, SCQ], bf16, tag="xb")
                nc.sync.dma_start(nxb[:], xT_v[:, :, (s + 1) * SCQ:
                                                (s + 2) * SCQ])
                bq = b_thunks(s + 1, nxb)
            if s > 0:
                dq = d_thunks(s - 1)
            gcount = 0

            for h in range(HPC):
                if h == 0:
                    q_rows, k_rows = qt1[0:D, :], kt1[0:D, :]
                elif h == 1:
                    q_rows, k_rows = qt1[D:P, :], kt1[D:P, :]
                else:
                    q_rows, k_rows = qt2[:, :], kt2[:, :]
                q_ap = q_rows[:, s * SCQ:(s + 1) * SCQ]
                nkj = 4 * s + 4
                psy = y_ps.tile([P, 4 * VWS], f32, tag="y")
                for g0 in range(0, nkj, GK):
                    gsz = min(GK, nkj - g0)
                    pss = s_ps.tile([P, GK * SCQ], f32, tag="ss")
                    for j in range(gsz):
                        kj = g0 + j
                        nc.tensor.matmul(
                            pss[:, j * SCQ:(j + 1) * SCQ],
                            k_rows[:, kj * P:(kj + 1) * P], q_ap,
                            start=True, stop=True)
                    pt = ptp.tile([P, GK * SCQ], bf16, tag="pt")
                    nc.scalar.activation(pt[:, :gsz * SCQ],
                                         pss[:, :gsz * SCQ], EXP,
                                         scale=0.125)
                    for j in range(gsz):
                        kj = g0 + j
                        if kj >= 4 * s:
                            jc = slice(j * SCQ, (j + 1) * SCQ)
                            nc.vector.tensor_mul(pt[:, jc], pt[:, jc],
                                                 mk_sb[kj - 4 * s][:])
                    step_pipeline((psy, (pt, h, s, g0, gsz), g0 + GK >= nkj))
                    gcount += 1
                    if bq:
                        bq.pop(0)()
                    elif dq and gcount >= 2:
                        dq.pop(0)()
            for f in bq + dq:
                f()

        # drain the pipeline
        step_pipeline(None)
        if st["pend_tr"] is not None:
            emit_y_tr(st["pend_tr"])
            st["pend_tr"] = None
        for f in d_thunks(NSC - 1):
            f()

    nc.compile()
    return nc


def _rope_add(nc, pend_rope, cols):
    ct, wt, dst = pend_rope
    if isinstance(dst, tuple):  # (q2, k2) split across two 64-row tiles
        q2, k2 = dst
        nc.vector.tensor_add(q2[:, cols], ct[0:D, :], wt[0:D, :])
        nc.vector.tensor_add(k2[:, cols], ct[D:P, :], wt[D:P, :])
    else:
        nc.vector.tensor_add(dst[:, cols], ct[:], wt[:])


def _emit_proj(nc, s, wpA, wpB, yt1, yt2, g_ps, otp, out_d):
    cols = slice(s * SCQ, (s + 1) * SCQ)
    for m in range(C // P):
        pso = g_ps.tile([P, SCQ], f32, tag="ps")
        nc.tensor.matmul(pso[:], wpA[:, m * P:(m + 1) * P], yt1[:, cols],
                         start=True, stop=False)
        nc.tensor.matmul(pso[:], wpB[:, m * P:(m + 1) * P], yt2[:, cols],
                         start=False, stop=True)
        ot = otp.tile([P, SCQ], f32, tag="ot")
        nc.vector.tensor_copy(ot[:], pso[:])
        nc.sync.dma_start(out_d[m * P:(m + 1) * P, cols], ot[:])


# ---------------------------------------------------------------------------
# host side
# ---------------------------------------------------------------------------

def make_core_inputs(x, Wq, bq, Wk, bk, Wv, bv, Wp, bp, T=4096, n_cores=8):
    """Build the per-core input maps (bf16 device tensors). Biases bq/bk must
    be zero; bv/bp are folded on the host in run()."""
    cpat = np.empty((P, T), dtype=np.float32)
    spat = np.empty((P, T), dtype=np.float32)
    inv_freq = (10000.0 ** (-(np.arange(32, dtype=np.float64)) / 32.0))
    ang = np.arange(T, dtype=np.float64)[None, :] * inv_freq[:, None]  # [32,T]
    cos32 = np.cos(ang).astype(np.float32)
    sin32 = np.sin(ang).astype(np.float32)
    for blk in range(4):
        cpat[blk * 32:(blk + 1) * 32] = cos32
        spat[blk * 32:(blk + 1) * 32] = sin32 if blk % 2 == 0 else -sin32

    jj = np.arange(P)[:, None]
    ii = np.arange(SCQ)[None, :]
    mks = [(jj + P * d <= ii).astype(np.float32) for d in range(4)]

    in_maps = []
    for c in range(n_cores):
        b, g = divmod(c, n_cores // 2)
        heads = [HPC * g + i for i in range(HPC)]

        def eo(h):  # [even d | odd d] rows of head h
            base = D * h
            return [base + 2 * i for i in range(32)] + \
                   [base + 2 * i + 1 for i in range(32)]

        v_rows = []
        for h in heads:
            v_rows += list(range(D * h, D * (h + 1)))
        # m-chunks: [q01 | k01 | q2+k2 | v]
        w_cat = np.concatenate(
            [Wq[eo(heads[0]) + eo(heads[1]), :].T,
             Wk[eo(heads[0]) + eo(heads[1]), :].T,
             Wq[eo(heads[2]), :].T, Wk[eo(heads[2]), :].T,
             Wv[v_rows, :].T],
            axis=1)
        wp_s = np.ascontiguousarray(Wp[:, v_rows].T).astype(BF)
        xT = np.ascontiguousarray(x[b].T).astype(BF)
        im = {
            "xT": xT, "w": np.ascontiguousarray(w_cat).astype(BF),
            "wp": wp_s,
            "cpat": cpat.astype(BF), "spat": spat.astype(BF),
        }
        for d in range(4):
            im[f"mk{d}"] = mks[d].astype(BF)
        in_maps.append(im)
    return in_maps


_nc_cache = {}


def run(x, Wq, bq, Wk, bk, Wv, bv, Wp, bp, T=4096, n_cores=8, trace=False,
        trace_cores=None):
    assert not (np.any(bq) or np.any(bk)), "nonzero q/k bias unsupported"
    key = (T, n_cores)
    if key not in _nc_cache:
        _nc_cache[key] = build(T=T, n_cores=n_cores)
    nc = _nc_cache[key]
    in_maps = make_core_inputs(x, Wq, bq, Wk, bk, Wv, bv, Wp, bp,
                               T=T, n_cores=n_cores)
    res = run_bass_kernel_spmd(nc, in_maps, list(range(n_cores)), trace=trace,
                               trace_cores=trace_cores)
    B = 2
    out = np.zeros((B, T, C), dtype=np.float32)
    for c in range(n_cores):
        b = c // (n_cores // 2)
        out[b] += np.asarray(res.results[c]["outT"], dtype=np.float32).T
    # host-folded bias terms: softmax rows sum to 1, so the v bias passes
    # through attention unchanged: y = att@v + bv  =>  out += bv @ Wp.T + bp
    out += (bv.astype(np.float32) @ Wp.T.astype(np.float32) + bp)[None, None, :]
    return out, res


def kernel(**inputs):
    inputs = {k: np.asarray(v) for k, v in inputs.items()}
    out, _ = run(**inputs)
    return out


# revision 23
# speedup vs baseline: 1.1667x; 1.0901x over previous
"""Causal self-attention (B=2, T=4096, C=768, H=12, D=64, RoPE) on 8 TRN2 cores.

Sharding: core c handles batch b = c//4 and heads [3g, 3g+1, 3g+2] with g = c%4
(data parallel over B, tensor parallel over heads). Each core computes its
heads' QKV projections, RoPE, causal attention and the partial output
projection; the host sums the 4 partial projections per batch.

All device activations/weights are bf16 (pre-cast on the host); PSUM stays
fp32.  The kernel runs a single wavefront over 512-column superchunks so the
tensor engine is continuously fed (QKV projection, attention and output
projection for successive superchunks interleave):

  for s: B(s) = QKV+rope+v^T for cols [512s,512s+512)
         D(s-1) = output projection for the previous superchunk
         C(s)  = attention for query superchunk s, all 3 heads

Device-side layout (per core):
  - x ships transposed bf16: xT [768, 4096].
  - q/k are channel-major in three 128-row m-chunks (q01 | k01 | q2+k2),
    each head's channels permuted [even|odd] so RoPE is two elementwise
    muls, a 32-partition swap (SBUF->SBUF DMA) and an add.
  - v^T is computed directly (x^T chunks stationary, Wv moving) into
    v_aug key-major tiles with a ones column per head (65-wide slots) so
    PV accumulates the softmax denominator for free.
  - S^T = k-blocks^T q lands in PSUM; exp runs on the scalar engine
    (PSUM -> SBUF bf16); diagonal blocks get a bf16 mask multiply (DVE).
  - PV is transposed: P^T 128-query chunks are stationary, v_aug moving;
    y arrives [queries, channels] with a per-query denominator, normalized
    by gpsimd.normalize_recip and transposed back to channel-major on the
    tensor engine.
  - the output projection emits outT [768, 4096] fp32 partials.
"""

import sys

sys.path.insert(0, "/opt/trn_rl_repo")

from contextlib import ExitStack

import numpy as np
import ml_dtypes

import concourse.bass as bass
import concourse.tile as tile
from concourse import bacc, mybir
from concourse.bass_utils import run_bass_kernel_spmd
from concourse.masks import make_identity

P = 128
C = 768
D = 64
HPC = 3            # heads per core
DQ = HPC * D       # 192 channels per core
W3 = 3 * DQ        # 576 = q+k+v output channels per core
KCH = C // P       # 6 contraction chunks for projections
SCQ = 512          # superchunk (phase B cols == query block == proj cols)
GK = 2             # S^T tiles per exp group
VWS = 128          # v_aug slot width (64 v + ones + zero pad = PE filler)
VW = HPC * VWS     # v_aug row width per key chunk

f32 = mybir.dt.float32
bf16 = mybir.dt.bfloat16
EXP = mybir.ActivationFunctionType.Exp
BF = ml_dtypes.bfloat16


def build(T=4096, n_cores=8):
    NSC = T // SCQ
    nc = bacc.Bacc("TRN2", target_bir_lowering=False, debug=False,
                   num_devices=n_cores)

    xT_d = nc.dram_tensor("xT", [C, T], bf16, kind="ExternalInput").ap()
    w_d = nc.dram_tensor("w", [C, W3], bf16, kind="ExternalInput").ap()
    wp_d = nc.dram_tensor("wp", [DQ, C], bf16, kind="ExternalInput").ap()
    cp_d = nc.dram_tensor("cpat", [P, T], bf16, kind="ExternalInput").ap()
    sp_d = nc.dram_tensor("spat", [P, T], bf16, kind="ExternalInput").ap()
    mk_d = [nc.dram_tensor(f"mk{d}", [P, SCQ], bf16, kind="ExternalInput").ap()
            for d in range(4)]
    out_d = nc.dram_tensor("outT", [C, T], f32, kind="ExternalOutput").ap()

    xT_v = xT_d.rearrange("(a p) t -> p a t", p=P)
    w_v = w_d.rearrange("(a p) n -> p a n", p=P)

    with tile.TileContext(nc) as tc, ExitStack() as top:
        const = top.enter_context(tc.tile_pool(name="const", bufs=1))
        persist = top.enter_context(tc.tile_pool(name="persist", bufs=1))
        xp = top.enter_context(tc.tile_pool(name="xp", bufs=2))
        rtmp = top.enter_context(tc.tile_pool(name="rtmp", bufs=2))
        ptp = top.enter_context(tc.tile_pool(name="ptp", bufs=3))
        smp = top.enter_context(tc.tile_pool(name="smp", bufs=2))
        otp = top.enter_context(tc.tile_pool(name="otp", bufs=3))
        # PSUM: shared [128,512] ring (QKV / v^T / proj) + attention pools
        g_ps = top.enter_context(tc.tile_pool(name="g_ps", bufs=2,
                                              space="PSUM"))
        s_ps = top.enter_context(tc.tile_pool(name="s_ps", bufs=2,
                                              space="PSUM"))
        y_ps = top.enter_context(tc.tile_pool(name="y_ps", bufs=1,
                                              space="PSUM"))
        tr_ps = top.enter_context(tc.tile_pool(name="tr_ps", bufs=1,
                                               space="PSUM"))

        # --- constants ---
        w_r = const.tile([P, KCH, W3], bf16)
        wpA = const.tile([P, C], bf16)
        wpB = const.tile([DQ - P, C], bf16)
        cp_sb = const.tile([P, T], bf16)
        sp_sb = const.tile([P, T], bf16)
        mk_sb = [const.tile([P, SCQ], bf16, tag=f"mk{d}", name=f"mk{d}")
                 for d in range(4)]
        ident = const.tile([P, P], bf16)

        # persistent activations
        qt1 = persist.tile([P, T], bf16)
        kt1 = persist.tile([P, T], bf16)
        qt2 = persist.tile([D, T], bf16)
        kt2 = persist.tile([D, T], bf16)
        v_aug = persist.tile([P, (T // P) * VW], bf16)
        yt1 = persist.tile([P, T], bf16)
        yt2 = persist.tile([D, T], bf16)
        v4 = v_aug[:].rearrange("p (a h c) -> p a h c", h=HPC, c=VWS)

        # preamble: xb(0) first on sync so the PE can start ASAP; the
        # remaining constants go behind it / on the scalar queue.
        xb0 = xp.tile([P, KCH, SCQ], bf16, tag="xb")
        nc.sync.dma_start(xb0[:], xT_v[:, :, 0:SCQ])
        nc.scalar.dma_start(w_r[:], w_v[:])
        nc.sync.dma_start(cp_sb[:], cp_d[:])
        nc.scalar.dma_start(sp_sb[:], sp_d[:])
        for d in range(4):
            (nc.sync if d % 2 else nc.scalar).dma_start(mk_sb[d][:],
                                                        mk_d[d][:])
        nc.sync.dma_start(wpA[:], wp_d[0:P, :])
        nc.scalar.dma_start(wpB[:], wp_d[P:DQ, :])
        make_identity(nc, ident[:])
        nc.gpsimd.memset(v_aug[:], 0.0)
        nc.gpsimd.memset(v4[:, :, :, 64], 1.0)
        # prefetch the exp activation table so the first real exp is cheap
        warm = const.tile([1, 8], f32)
        nc.scalar.activation(warm[:], ident[0:1, 0:8], EXP, scale=1.0)

        st = {"pend_pv": None, "pend_tr": None, "xb": xb0}

        def emit_pv(psy, pend):
            pt, h, s, g0, gsz = pend
            nkj = 4 * s + 4
            for j in range(gsz):
                kj = g0 + j
                va = v_aug[:, kj * VW + h * VWS: kj * VW + (h + 1) * VWS]
                for qc in range(4):
                    nc.tensor.matmul(
                        psy[:, qc * VWS:(qc + 1) * VWS],
                        pt[:, j * SCQ + qc * P: j * SCQ + (qc + 1) * P],
                        va,
                        start=(kj == 0 and qc == 0),
                        stop=(kj == nkj - 1 and qc == 3))

        def emit_y_norm(psy, h, s):
            """copy + per-query normalize (DVE + Pool only, no PE)."""
            ysb = smp.tile([P, 4 * VWS], f32, tag="ysb")
            nc.vector.tensor_copy(ysb[:], psy[:])
            yn = smp.tile([P, 4 * D], bf16, tag="yn")
            for qc in range(4):
                nc.gpsimd.normalize_recip(
                    yn[:, qc * D:(qc + 1) * D],
                    ysb[:, qc * VWS:qc * VWS + D],
                    ysb[:, qc * VWS + D:qc * VWS + D + 1])
            return yn

        def emit_y_tr(pend_tr):
            """transpose y back to channel-major and store (PE + one copy)."""
            yn, h, s = pend_tr
            ytr = tr_ps.tile([D, 4 * P], bf16, tag="tr")
            for qc in range(4):
                nc.tensor.transpose(ytr[:, qc * P:(qc + 1) * P],
                                    yn[:, qc * D:(qc + 1) * D], ident[:])
            scols = slice(s * SCQ, (s + 1) * SCQ)
            ydst = yt1[h * D:(h + 1) * D, scols] if h < 2 else yt2[:, scols]
            nc.vector.tensor_copy(ydst, ytr[:])

        def step_pipeline(new_pend):
            """emit the pending PV; the delayed transpose of the unit before
            last; and, when the pending PV closes a unit, its normalize.
            (every unit has >= 2 groups, so pend_tr always drains before the
            next unit closes)"""
            if st["pend_pv"] is not None:
                ppsy, pend, last = st["pend_pv"]
                emit_pv(ppsy, pend)
                if st["pend_tr"] is not None:
                    emit_y_tr(st["pend_tr"])
                    st["pend_tr"] = None
                if last:
                    yn = emit_y_norm(ppsy, pend[1], pend[2])
                    st["pend_tr"] = (yn, pend[1], pend[2])
            st["pend_pv"] = new_pend

        def b_thunks(s, xb):
            """Superchunk s's QKV+rope+v^T as 7 closures (3 m-chunks + 4
            v^T key-blocks), to be sprinkled between attention groups."""
            cols = slice(s * SCQ, (s + 1) * SCQ)
            bst = {"pend": None}

            def mk_m(mi, dst):
                def f():
                    ps = g_ps.tile([P, SCQ], f32, tag="ps")
                    for kc in range(KCH):
                        nc.tensor.matmul(
                            ps[:], w_r[:, kc, mi * P:(mi + 1) * P],
                            xb[:, kc, :],
                            start=(kc == 0), stop=(kc == KCH - 1))
                    stt = rtmp.tile([P, SCQ], bf16, tag="st")
                    nc.vector.tensor_mul(stt[:], ps[:], sp_sb[:, cols])
                    ct = rtmp.tile([P, SCQ], bf16, tag="ct")
                    nc.vector.tensor_mul(ct[:], ps[:], cp_sb[:, cols])
                    wt = rtmp.tile([P, SCQ], bf16, tag="wt")
                    for p0 in (0, D):
                        nc.sync.dma_start(wt[p0:p0 + 32, :],
                                          stt[p0 + 32:p0 + D, :])
                        nc.sync.dma_start(wt[p0 + 32:p0 + D, :],
                                          stt[p0:p0 + 32, :])
                    if bst["pend"] is not None:
                        _rope_add(nc, bst["pend"], cols)
                    bst["pend"] = (ct, wt, dst)
                return f

            def mk_v(kb):
                def f():
                    psv = g_ps.tile([P, SCQ], f32, tag="ps")
                    for kc in range(KCH):
                        nc.tensor.matmul(
                            psv[:, 0:DQ], xb[:, kc, kb * P:(kb + 1) * P],
                            w_r[:, kc, 2 * DQ:W3],
                            start=(kc == 0), stop=(kc == KCH - 1))
                    if bst["pend"] is not None:
                        _rope_add(nc, bst["pend"], cols)
                        bst["pend"] = None
                    kc32 = s * (SCQ // P) + kb
                    nc.vector.tensor_copy(
                        v4[:, kc32, :, 0:D],
                        psv[:, 0:DQ].rearrange("p (h c) -> p h c", h=HPC))
                return f

            return [mk_m(0, qt1), mk_m(1, kt1), mk_m(2, (qt2, kt2)),
                    mk_v(0), mk_v(1), mk_v(2), mk_v(3)]

        def d_thunks(s):
            """Output projection for superchunk s as 6 closures."""
            cols = slice(s * SCQ, (s + 1) * SCQ)

            def mk(m):
                def f():
                    pso = g_ps.tile([P, SCQ], f32, tag="ps")
                    nc.tensor.matmul(pso[:], wpA[:, m * P:(m + 1) * P],
                                     yt1[:, cols], start=True, stop=False)
                    nc.tensor.matmul(pso[:], wpB[:, m * P:(m + 1) * P],
                                     yt2[:, cols], start=False, stop=True)
                    ot = otp.tile([P, SCQ], f32, tag="ot")
                    nc.vector.tensor_copy(ot[:], pso[:])
                    nc.sync.dma_start(out_d[m * P:(m + 1) * P, cols], ot[:])
                return f

            return [mk(m) for m in range(C // P)]

        # B(0) runs standalone (C(0) needs it immediately)
        for f in b_thunks(0, xb0):
            f()

        for s in range(NSC):
            # filler for this iteration: B(s+1) pieces first; the output
            # projections are all deferred to the last two iterations, whose
            # attention units are the largest and would otherwise starve the
            # PE (no B work left there).
            bq, dq = [], []
            if s + 1 < NSC:
                nxb = xp.tile([P, KCH, SCQ], bf16, tag="xb")
                nc.sync.dma_start(nxb[:], xT_v[:, :, (s + 1) * SCQ:
                                                (s + 2) * SCQ])
                bq = b_thunks(s + 1, nxb)
            if NSC >= 4 and s == NSC - 2:
                dq = [f for u in range(0, 3) for f in d_thunks(u)]
            elif s == NSC - 1:
                lo = 3 if NSC >= 4 else 0
                dq = [f for u in range(lo, NSC - 1) for f in d_thunks(u)]
            gcount = 0

            for h in range(HPC):
                if h == 0:
                    q_rows, k_rows = qt1[0:D, :], kt1[0:D, :]
                elif h == 1:
                    q_rows, k_rows = qt1[D:P, :], kt1[D:P, :]
                else:
                    q_rows, k_rows = qt2[:, :], kt2[:, :]
                q_ap = q_rows[:, s * SCQ:(s + 1) * SCQ]
                nkj = 4 * s + 4
                psy = y_ps.tile([P, 4 * VWS], f32, tag="y")
                for g0 in range(0, nkj, GK):
                    gsz = min(GK, nkj - g0)
                    pss = s_ps.tile([P, GK * SCQ], f32, tag="ss")
                    for j in range(gsz):
                        kj = g0 + j
                        nc.tensor.matmul(
                            pss[:, j * SCQ:(j + 1) * SCQ],
                            k_rows[:, kj * P:(kj + 1) * P], q_ap,
                            start=True, stop=True)
                    pt = ptp.tile([P, GK * SCQ], bf16, tag="pt")
                    nc.scalar.activation(pt[:, :gsz * SCQ],
                                         pss[:, :gsz * SCQ], EXP,
                                         scale=0.125)
                    for j in range(gsz):
                        kj = g0 + j
                        if kj >= 4 * s:
                            jc = slice(j * SCQ, (j + 1) * SCQ)
                            nc.vector.tensor_mul(pt[:, jc], pt[:, jc],
                                                 mk_sb[kj - 4 * s][:])
                    step_pipeline((psy, (pt, h, s, g0, gsz), g0 + GK >= nkj))
                    gcount += 1
                    if bq:
                        bq.pop(0)()
                    elif dq and gcount >= 2:
                        # >= 2: D(s-1) needs the y of (h2, s-1), emitted
                        # during this iteration's second group
                        dq.pop(0)()
            for f in bq + dq:
                f()

        # drain the pipeline
        step_pipeline(None)
        if st["pend_tr"] is not None:
            emit_y_tr(st["pend_tr"])
            st["pend_tr"] = None
        for f in d_thunks(NSC - 1):
            f()

    nc.compile()
    return nc


def _rope_add(nc, pend_rope, cols):
    ct, wt, dst = pend_rope
    if isinstance(dst, tuple):  # (q2, k2) split across two 64-row tiles
        q2, k2 = dst
        nc.vector.tensor_add(q2[:, cols], ct[0:D, :], wt[0:D, :])
        nc.vector.tensor_add(k2[:, cols], ct[D:P, :], wt[D:P, :])
    else:
        nc.vector.tensor_add(dst[:, cols], ct[:], wt[:])


def _emit_proj(nc, s, wpA, wpB, yt1, yt2, g_ps, otp, out_d):
    cols = slice(s * SCQ, (s + 1) * SCQ)
    for m in range(C // P):
        pso = g_ps.tile([P, SCQ], f32, tag="ps")
        nc.tensor.matmul(pso[:], wpA[:, m * P:(m + 1) * P], yt1[:, cols],
                         start=True, stop=False)
        nc.tensor.matmul(pso[:], wpB[:, m * P:(m + 1) * P], yt2[:, cols],
                         start=False, stop=True)
        ot = otp.tile([P, SCQ], f32, tag="ot")
        nc.vector.tensor_copy(ot[:], pso[:])
        nc.sync.dma_start(out_d[m * P:(m + 1) * P, cols], ot[:])


# ---------------------------------------------------------------------------
# host side
# ---------------------------------------------------------------------------

def make_core_inputs(x, Wq, bq, Wk, bk, Wv, bv, Wp, bp, T=4096, n_cores=8):
    """Build the per-core input maps (bf16 device tensors). Biases bq/bk must
    be zero; bv/bp are folded on the host in run()."""
    cpat = np.empty((P, T), dtype=np.float32)
    spat = np.empty((P, T), dtype=np.float32)
    inv_freq = (10000.0 ** (-(np.arange(32, dtype=np.float64)) / 32.0))
    ang = np.arange(T, dtype=np.float64)[None, :] * inv_freq[:, None]  # [32,T]
    cos32 = np.cos(ang).astype(np.float32)
    sin32 = np.sin(ang).astype(np.float32)
    for blk in range(4):
        cpat[blk * 32:(blk + 1) * 32] = cos32
        spat[blk * 32:(blk + 1) * 32] = sin32 if blk % 2 == 0 else -sin32

    jj = np.arange(P)[:, None]
    ii = np.arange(SCQ)[None, :]
    mks = [(jj + P * d <= ii).astype(np.float32) for d in range(4)]

    in_maps = []
    for c in range(n_cores):
        b, g = divmod(c, n_cores // 2)
        heads = [HPC * g + i for i in range(HPC)]

        def eo(h):  # [even d | odd d] rows of head h
            base = D * h
            return [base + 2 * i for i in range(32)] + \
                   [base + 2 * i + 1 for i in range(32)]

        v_rows = []
        for h in heads:
            v_rows += list(range(D * h, D * (h + 1)))
        # m-chunks: [q01 | k01 | q2+k2 | v]
        w_cat = np.concatenate(
            [Wq[eo(heads[0]) + eo(heads[1]), :].T,
             Wk[eo(heads[0]) + eo(heads[1]), :].T,
             Wq[eo(heads[2]), :].T, Wk[eo(heads[2]), :].T,
             Wv[v_rows, :].T],
            axis=1)
        wp_s = np.ascontiguousarray(Wp[:, v_rows].T).astype(BF)
        xT = np.ascontiguousarray(x[b].T).astype(BF)
        im = {
            "xT": xT, "w": np.ascontiguousarray(w_cat).astype(BF),
            "wp": wp_s,
            "cpat": cpat.astype(BF), "spat": spat.astype(BF),
        }
        for d in range(4):
            im[f"mk{d}"] = mks[d].astype(BF)
        in_maps.append(im)
    return in_maps


_nc_cache = {}


def run(x, Wq, bq, Wk, bk, Wv, bv, Wp, bp, T=4096, n_cores=8, trace=False,
        trace_cores=None):
    assert not (np.any(bq) or np.any(bk)), "nonzero q/k bias unsupported"
    key = (T, n_cores)
    if key not in _nc_cache:
        _nc_cache[key] = build(T=T, n_cores=n_cores)
    nc = _nc_cache[key]
    in_maps = make_core_inputs(x, Wq, bq, Wk, bk, Wv, bv, Wp, bp,
                               T=T, n_cores=n_cores)
    res = run_bass_kernel_spmd(nc, in_maps, list(range(n_cores)), trace=trace,
                               trace_cores=trace_cores)
    B = 2
    out = np.zeros((B, T, C), dtype=np.float32)
    for c in range(n_cores):
        b = c // (n_cores // 2)
        out[b] += np.asarray(res.results[c]["outT"], dtype=np.float32).T
    # host-folded bias terms: softmax rows sum to 1, so the v bias passes
    # through attention unchanged: y = att@v + bv  =>  out += bv @ Wp.T + bp
    out += (bv.astype(np.float32) @ Wp.T.astype(np.float32) + bp)[None, None, :]
    return out, res


def kernel(**inputs):
    inputs = {k: np.asarray(v) for k, v in inputs.items()}
    out, _ = run(**inputs)
    return out


# revision 28
# speedup vs baseline: 1.1858x; 1.0164x over previous
"""Causal self-attention (B=2, T=4096, C=768, H=12, D=64, RoPE) on 8 TRN2 cores.

Sharding: core c handles batch b = c//4 and heads [3g, 3g+1, 3g+2] with g = c%4
(data parallel over B, tensor parallel over heads). Each core computes its
heads' QKV projections, RoPE, causal attention and the partial output
projection; the host sums the 4 partial projections per batch.

All device activations/weights are bf16 (pre-cast on the host); PSUM stays
fp32.  The kernel runs a single wavefront over 512-column superchunks so the
tensor engine is continuously fed (QKV projection, attention and output
projection for successive superchunks interleave):

  for s: B(s) = QKV+rope+v^T for cols [512s,512s+512)
         D(s-1) = output projection for the previous superchunk
         C(s)  = attention for query superchunk s, all 3 heads

Device-side layout (per core):
  - x ships transposed bf16: xT [768, 4096].
  - q/k are channel-major in three 128-row m-chunks (q01 | k01 | q2+k2),
    each head's channels permuted [even|odd] so RoPE is two elementwise
    muls, a 32-partition swap (SBUF->SBUF DMA) and an add.
  - v^T is computed directly (x^T chunks stationary, Wv moving) into
    v_aug key-major tiles with a ones column per head (65-wide slots) so
    PV accumulates the softmax denominator for free.
  - S^T = k-blocks^T q lands in PSUM; exp runs on the scalar engine
    (PSUM -> SBUF bf16); diagonal blocks get a bf16 mask multiply (DVE).
  - PV is transposed: P^T 128-query chunks are stationary, v_aug moving;
    y arrives [queries, channels] with a per-query denominator, normalized
    by gpsimd.normalize_recip and transposed back to channel-major on the
    tensor engine.
  - the output projection emits outT [768, 4096] fp32 partials.
"""

import sys

sys.path.insert(0, "/opt/trn_rl_repo")

from contextlib import ExitStack

import numpy as np
import ml_dtypes

import concourse.bass as bass
import concourse.tile as tile
from concourse import bacc, mybir
from concourse.bass_utils import run_bass_kernel_spmd
from concourse.masks import make_identity

P = 128
C = 768
D = 64
HPC = 3            # heads per core
DQ = HPC * D       # 192 channels per core
W3 = 3 * DQ        # 576 = q+k+v output channels per core
KCH = C // P       # 6 contraction chunks for projections
SCQ = 512          # superchunk (phase B cols == query block == proj cols)
GK = 2             # S^T tiles per exp group
VWS = 128          # v_aug slot width (64 v + ones + zero pad = PE filler)
VW = HPC * VWS     # v_aug row width per key chunk

f32 = mybir.dt.float32
bf16 = mybir.dt.bfloat16
EXP = mybir.ActivationFunctionType.Exp
BF = ml_dtypes.bfloat16


def build(T=4096, n_cores=8):
    NSC = T // SCQ
    nc = bacc.Bacc("TRN2", target_bir_lowering=False, debug=False,
                   num_devices=n_cores)

    xT_d = nc.dram_tensor("xT", [C, T], bf16, kind="ExternalInput").ap()
    w_d = nc.dram_tensor("w", [C, W3], bf16, kind="ExternalInput").ap()
    wp_d = nc.dram_tensor("wp", [DQ, C], bf16, kind="ExternalInput").ap()
    cp_d = nc.dram_tensor("cpat", [P, T], bf16, kind="ExternalInput").ap()
    sp_d = nc.dram_tensor("spat", [P, T], bf16, kind="ExternalInput").ap()
    mk_d = [nc.dram_tensor(f"mk{d}", [P, SCQ], bf16, kind="ExternalInput").ap()
            for d in range(4)]
    out_d = nc.dram_tensor("outT", [C, T], f32, kind="ExternalOutput").ap()

    xT_v = xT_d.rearrange("(a p) t -> p a t", p=P)
    w_v = w_d.rearrange("(a p) n -> p a n", p=P)

    with tile.TileContext(nc) as tc, ExitStack() as top:
        const = top.enter_context(tc.tile_pool(name="const", bufs=1))
        persist = top.enter_context(tc.tile_pool(name="persist", bufs=1))
        xp = top.enter_context(tc.tile_pool(name="xp", bufs=2))
        rtmp = top.enter_context(tc.tile_pool(name="rtmp", bufs=2))
        ptp = top.enter_context(tc.tile_pool(name="ptp", bufs=3))
        smp = top.enter_context(tc.tile_pool(name="smp", bufs=2))
        otp = top.enter_context(tc.tile_pool(name="otp", bufs=3))
        # PSUM: shared [128,512] ring (QKV / v^T / proj) + attention pools
        g_ps = top.enter_context(tc.tile_pool(name="g_ps", bufs=2,
                                              space="PSUM"))
        s_ps = top.enter_context(tc.tile_pool(name="s_ps", bufs=2,
                                              space="PSUM"))
        y_ps = top.enter_context(tc.tile_pool(name="y_ps", bufs=1,
                                              space="PSUM"))
        tr_ps = top.enter_context(tc.tile_pool(name="tr_ps", bufs=1,
                                               space="PSUM"))

        # --- constants ---
        w_r = const.tile([P, KCH, W3], bf16)
        wpA = const.tile([P, C], bf16)
        wpB = const.tile([DQ - P, C], bf16)
        cp_sb = const.tile([P, T], bf16)
        sp_sb = const.tile([P, T], bf16)
        mk_sb = [const.tile([P, SCQ], bf16, tag=f"mk{d}", name=f"mk{d}")
                 for d in range(4)]
        ident = const.tile([P, P], bf16)

        # persistent activations
        qt1 = persist.tile([P, T], bf16)
        kt1 = persist.tile([P, T], bf16)
        qt2 = persist.tile([D, T], bf16)
        kt2 = persist.tile([D, T], bf16)
        v_aug = persist.tile([P, (T // P) * VW], bf16)
        yt1 = persist.tile([P, T], bf16)
        yt2 = persist.tile([D, T], bf16)
        v4 = v_aug[:].rearrange("p (a h c) -> p a h c", h=HPC, c=VWS)

        # preamble: xb(0) first on sync so the PE can start ASAP; the
        # remaining constants go behind it / on the scalar queue.
        xb0 = xp.tile([P, KCH, SCQ], bf16, tag="xb")
        nc.sync.dma_start(xb0[:], xT_v[:, :, 0:SCQ])
        nc.scalar.dma_start(w_r[:], w_v[:])
        nc.sync.dma_start(cp_sb[:], cp_d[:])
        nc.scalar.dma_start(sp_sb[:], sp_d[:])
        for d in range(4):
            (nc.sync if d % 2 else nc.scalar).dma_start(mk_sb[d][:],
                                                        mk_d[d][:])
        nc.sync.dma_start(wpA[:], wp_d[0:P, :])
        nc.scalar.dma_start(wpB[:], wp_d[P:DQ, :])
        make_identity(nc, ident[:])
        nc.gpsimd.memset(v_aug[:], 0.0)
        nc.gpsimd.memset(v4[:, :, :, 64], 1.0)
        # prefetch the exp activation table so the first real exp is cheap
        warm = const.tile([1, 8], f32)
        nc.scalar.activation(warm[:], ident[0:1, 0:8], EXP, scale=1.0)

        st = {"pend_pv": None, "pend_tr": None, "xb": xb0}

        def emit_pv(psy, pend):
            pt, h, s, g0, gsz = pend
            nkj = 4 * s + 4
            for j in range(gsz):
                kj = g0 + j
                va = v_aug[:, kj * VW + h * VWS: kj * VW + (h + 1) * VWS]
                for qc in range(4):
                    nc.tensor.matmul(
                        psy[:, qc * VWS:(qc + 1) * VWS],
                        pt[:, j * SCQ + qc * P: j * SCQ + (qc + 1) * P],
                        va,
                        start=(kj == 0 and qc == 0),
                        stop=(kj == nkj - 1 and qc == 3))

        def emit_y_norm(psy, h, s):
            """copy + per-query normalize (DVE + Pool only, no PE)."""
            ysb = smp.tile([P, 4 * VWS], f32, tag="ysb")
            nc.vector.tensor_copy(ysb[:], psy[:])
            yn = smp.tile([P, 4 * D], bf16, tag="yn")
            for qc in range(4):
                nc.gpsimd.normalize_recip(
                    yn[:, qc * D:(qc + 1) * D],
                    ysb[:, qc * VWS:qc * VWS + D],
                    ysb[:, qc * VWS + D:qc * VWS + D + 1])
            return yn

        def emit_y_tr(pend_tr):
            """transpose y back to channel-major and store (PE + one copy)."""
            yn, h, s = pend_tr
            ytr = tr_ps.tile([D, 4 * P], bf16, tag="tr")
            for qc in range(4):
                nc.tensor.transpose(ytr[:, qc * P:(qc + 1) * P],
                                    yn[:, qc * D:(qc + 1) * D], ident[:])
            scols = slice(s * SCQ, (s + 1) * SCQ)
            ydst = yt1[h * D:(h + 1) * D, scols] if h < 2 else yt2[:, scols]
            nc.vector.tensor_copy(ydst, ytr[:])

        def step_pipeline(new_pend):
            """emit the pending PV; the delayed transpose of the unit before
            last; and, when the pending PV closes a unit, its normalize.
            (every unit has >= 2 groups, so pend_tr always drains before the
            next unit closes)"""
            if st["pend_pv"] is not None:
                ppsy, pend, last = st["pend_pv"]
                emit_pv(ppsy, pend)
                if st["pend_tr"] is not None:
                    emit_y_tr(st["pend_tr"])
                    st["pend_tr"] = None
                if last:
                    yn = emit_y_norm(ppsy, pend[1], pend[2])
                    st["pend_tr"] = (yn, pend[1], pend[2])
            st["pend_pv"] = new_pend

        def b_thunks(s, xb):
            """Superchunk s's QKV+rope+v^T as 7 closures (3 m-chunks + 4
            v^T key-blocks), to be sprinkled between attention groups."""
            cols = slice(s * SCQ, (s + 1) * SCQ)
            bst = {"pend": None}

            def mk_m(mi, dst):
                def f():
                    ps = g_ps.tile([P, SCQ], f32, tag="ps")
                    for kc in range(KCH):
                        nc.tensor.matmul(
                            ps[:], w_r[:, kc, mi * P:(mi + 1) * P],
                            xb[:, kc, :],
                            start=(kc == 0), stop=(kc == KCH - 1))
                    stt = rtmp.tile([P, SCQ], bf16, tag="st")
                    nc.vector.tensor_mul(stt[:], ps[:], sp_sb[:, cols])
                    ct = rtmp.tile([P, SCQ], bf16, tag="ct")
                    nc.vector.tensor_mul(ct[:], ps[:], cp_sb[:, cols])
                    wt = rtmp.tile([P, SCQ], bf16, tag="wt")
                    for p0 in (0, D):
                        nc.sync.dma_start(wt[p0:p0 + 32, :],
                                          stt[p0 + 32:p0 + D, :])
                        nc.sync.dma_start(wt[p0 + 32:p0 + D, :],
                                          stt[p0:p0 + 32, :])
                    if bst["pend"] is not None:
                        _rope_add(nc, bst["pend"], cols)
                    bst["pend"] = (ct, wt, dst)
                return f

            def mk_v(kb):
                def f():
                    psv = g_ps.tile([P, SCQ], f32, tag="ps")
                    for kc in range(KCH):
                        nc.tensor.matmul(
                            psv[:, 0:DQ], xb[:, kc, kb * P:(kb + 1) * P],
                            w_r[:, kc, 2 * DQ:W3],
                            start=(kc == 0), stop=(kc == KCH - 1))
                    if bst["pend"] is not None:
                        _rope_add(nc, bst["pend"], cols)
                        bst["pend"] = None
                    kc32 = s * (SCQ // P) + kb
                    nc.vector.tensor_copy(
                        v4[:, kc32, :, 0:D],
                        psv[:, 0:DQ].rearrange("p (h c) -> p h c", h=HPC))
                return f

            return [mk_m(0, qt1), mk_m(1, kt1), mk_m(2, (qt2, kt2)),
                    mk_v(0), mk_v(1), mk_v(2), mk_v(3)]

        def d_thunks(s):
            """Output projection for superchunk s as 6 closures."""
            cols = slice(s * SCQ, (s + 1) * SCQ)

            def mk(m):
                def f():
                    pso = g_ps.tile([P, SCQ], f32, tag="ps")
                    nc.tensor.matmul(pso[:], wpA[:, m * P:(m + 1) * P],
                                     yt1[:, cols], start=True, stop=False)
                    nc.tensor.matmul(pso[:], wpB[:, m * P:(m + 1) * P],
                                     yt2[:, cols], start=False, stop=True)
                    ot = otp.tile([P, SCQ], f32, tag="ot")
                    nc.vector.tensor_copy(ot[:], pso[:])
                    nc.sync.dma_start(out_d[m * P:(m + 1) * P, cols], ot[:])
                return f

            return [mk(m) for m in range(C // P)]

        # xb prefetch queue, two superchunks deep
        xbq = {0: xb0}
        if NSC > 1:
            xbq[1] = xp.tile([P, KCH, SCQ], bf16, tag="xb", name="xb1")
            nc.sync.dma_start(xbq[1][:], xT_v[:, :, SCQ:2 * SCQ])

        # B(0): only q01/k01 (+ the v block that carries k01's rope add)
        # run inline; the rest interleaves into C(0) as filler.
        b0 = b_thunks(0, xb0)
        for f in (b0[0], b0[1], b0[3]):      # m0, m1, V0 (emits add(m1))
            f()
        b0_rest = [b0[4], b0[2], b0[5], b0[6]]   # V1, m2, V2, V3

        for s in range(NSC):
            if s + 2 < NSC:
                xbq[s + 2] = xp.tile([P, KCH, SCQ], bf16, tag="xb",
                                     name=f"xb{s + 2}")
                nc.sync.dma_start(xbq[s + 2][:],
                                  xT_v[:, :, (s + 2) * SCQ:(s + 3) * SCQ])
            # filler: leftover B(0) pieces (s=0), then B(s+1), then deferred
            # output projections (all packed into the last two iterations,
            # whose attention units are largest and have no B work left).
            fq0 = list(b0_rest) if s == 0 else []   # forced: 2 pops/step
            b0_rest = []
            bq = []
            if s + 1 < NSC:
                bq += b_thunks(s + 1, xbq[s + 1])
            dq = []
            if NSC >= 4 and s == NSC - 2:
                dq = [f for u in range(0, 3) for f in d_thunks(u)]
            elif s == NSC - 1:
                lo = 3 if NSC >= 4 else 0
                dq = [f for u in range(lo, NSC - 1) for f in d_thunks(u)]
            nb = len(bq)
            fq = bq + dq
            ngroups = 6 * (s + 1)
            gcount = 0
            popped = 0

            for h in range(HPC):
                if h == 0:
                    q_rows, k_rows = qt1[0:D, :], kt1[0:D, :]
                elif h == 1:
                    q_rows, k_rows = qt1[D:P, :], kt1[D:P, :]
                else:
                    q_rows, k_rows = qt2[:, :], kt2[:, :]
                q_ap = q_rows[:, s * SCQ:(s + 1) * SCQ]
                nkj = 4 * s + 4
                psy = y_ps.tile([P, 4 * VWS], f32, tag="y")
                for g0 in range(0, nkj, GK):
                    gsz = min(GK, nkj - g0)
                    pss = s_ps.tile([P, GK * SCQ], f32, tag="ss")
                    for j in range(gsz):
                        kj = g0 + j
                        nc.tensor.matmul(
                            pss[:, j * SCQ:(j + 1) * SCQ],
                            k_rows[:, kj * P:(kj + 1) * P], q_ap,
                            start=True, stop=True)
                    pt = ptp.tile([P, GK * SCQ], bf16, tag="pt")
                    nc.scalar.activation(pt[:, :gsz * SCQ],
                                         pss[:, :gsz * SCQ], EXP,
                                         scale=0.125)
                    for j in range(gsz):
                        kj = g0 + j
                        if kj >= 4 * s:
                            jc = slice(j * SCQ, (j + 1) * SCQ)
                            nc.vector.tensor_mul(pt[:, jc], pt[:, jc],
                                                 mk_sb[kj - 4 * s][:])
                    # proportionally-paced filler, BEFORE the PV emission so
                    # the pending exp gets extra latency slack.  D thunks may
                    # not pop before group 3 (the y of (h2, s-1) is emitted
                    # during this iteration's second group).
                    gcount += 1
                    if fq0:
                        for _ in range(2 if gcount == 1 else len(fq0)):
                            if fq0:
                                fq0.pop(0)()
                    target = min(len(fq), -(-gcount * len(fq) // ngroups))
                    while popped < target and not (popped >= nb
                                                   and gcount < 3):
                        fq[popped]()
                        popped += 1
                    step_pipeline((psy, (pt, h, s, g0, gsz), g0 + GK >= nkj))
            while popped < len(fq):
                fq[popped]()
                popped += 1

        # drain the pipeline
        step_pipeline(None)
        if st["pend_tr"] is not None:
            emit_y_tr(st["pend_tr"])
            st["pend_tr"] = None
        for f in d_thunks(NSC - 1):
            f()

    nc.compile()
    return nc


def _rope_add(nc, pend_rope, cols):
    ct, wt, dst = pend_rope
    if isinstance(dst, tuple):  # (q2, k2) split across two 64-row tiles
        q2, k2 = dst
        nc.vector.tensor_add(q2[:, cols], ct[0:D, :], wt[0:D, :])
        nc.vector.tensor_add(k2[:, cols], ct[D:P, :], wt[D:P, :])
    else:
        nc.vector.tensor_add(dst[:, cols], ct[:], wt[:])


def _emit_proj(nc, s, wpA, wpB, yt1, yt2, g_ps, otp, out_d):
    cols = slice(s * SCQ, (s + 1) * SCQ)
    for m in range(C // P):
        pso = g_ps.tile([P, SCQ], f32, tag="ps")
        nc.tensor.matmul(pso[:], wpA[:, m * P:(m + 1) * P], yt1[:, cols],
                         start=True, stop=False)
        nc.tensor.matmul(pso[:], wpB[:, m * P:(m + 1) * P], yt2[:, cols],
                         start=False, stop=True)
        ot = otp.tile([P, SCQ], f32, tag="ot")
        nc.vector.tensor_copy(ot[:], pso[:])
        nc.sync.dma_start(out_d[m * P:(m + 1) * P, cols], ot[:])


# ---------------------------------------------------------------------------
# host side
# ---------------------------------------------------------------------------

def make_core_inputs(x, Wq, bq, Wk, bk, Wv, bv, Wp, bp, T=4096, n_cores=8):
    """Build the per-core input maps (bf16 device tensors). Biases bq/bk must
    be zero; bv/bp are folded on the host in run()."""
    cpat = np.empty((P, T), dtype=np.float32)
    spat = np.empty((P, T), dtype=np.float32)
    inv_freq = (10000.0 ** (-(np.arange(32, dtype=np.float64)) / 32.0))
    ang = np.arange(T, dtype=np.float64)[None, :] * inv_freq[:, None]  # [32,T]
    cos32 = np.cos(ang).astype(np.float32)
    sin32 = np.sin(ang).astype(np.float32)
    for blk in range(4):
        cpat[blk * 32:(blk + 1) * 32] = cos32
        spat[blk * 32:(blk + 1) * 32] = sin32 if blk % 2 == 0 else -sin32

    jj = np.arange(P)[:, None]
    ii = np.arange(SCQ)[None, :]
    mks = [(jj + P * d <= ii).astype(np.float32) for d in range(4)]

    in_maps = []
    for c in range(n_cores):
        b, g = divmod(c, n_cores // 2)
        heads = [HPC * g + i for i in range(HPC)]

        def eo(h):  # [even d | odd d] rows of head h
            base = D * h
            return [base + 2 * i for i in range(32)] + \
                   [base + 2 * i + 1 for i in range(32)]

        v_rows = []
        for h in heads:
            v_rows += list(range(D * h, D * (h + 1)))
        # m-chunks: [q01 | k01 | q2+k2 | v]
        w_cat = np.concatenate(
            [Wq[eo(heads[0]) + eo(heads[1]), :].T,
             Wk[eo(heads[0]) + eo(heads[1]), :].T,
             Wq[eo(heads[2]), :].T, Wk[eo(heads[2]), :].T,
             Wv[v_rows, :].T],
            axis=1)
        wp_s = np.ascontiguousarray(Wp[:, v_rows].T).astype(BF)
        xT = np.ascontiguousarray(x[b].T).astype(BF)
        im = {
            "xT": xT, "w": np.ascontiguousarray(w_cat).astype(BF),
            "wp": wp_s,
            "cpat": cpat.astype(BF), "spat": spat.astype(BF),
        }
        for d in range(4):
            im[f"mk{d}"] = mks[d].astype(BF)
        in_maps.append(im)
    return in_maps


_nc_cache = {}


def run(x, Wq, bq, Wk, bk, Wv, bv, Wp, bp, T=4096, n_cores=8, trace=False,
        trace_cores=None):
    assert not (np.any(bq) or np.any(bk)), "nonzero q/k bias unsupported"
    key = (T, n_cores)
    if key not in _nc_cache:
        _nc_cache[key] = build(T=T, n_cores=n_cores)
    nc = _nc_cache[key]
    in_maps = make_core_inputs(x, Wq, bq, Wk, bk, Wv, bv, Wp, bp,
                               T=T, n_cores=n_cores)
    res = run_bass_kernel_spmd(nc, in_maps, list(range(n_cores)), trace=trace,
                               trace_cores=trace_cores)
    B = 2
    out = np.zeros((B, T, C), dtype=np.float32)
    for c in range(n_cores):
        b = c // (n_cores // 2)
        out[b] += np.asarray(res.results[c]["outT"], dtype=np.float32).T
    # host-folded bias terms: softmax rows sum to 1, so the v bias passes
    # through attention unchanged: y = att@v + bv  =>  out += bv @ Wp.T + bp
    out += (bv.astype(np.float32) @ Wp.T.astype(np.float32) + bp)[None, None, :]
    return out, res


def kernel(**inputs):
    inputs = {k: np.asarray(v) for k, v in inputs.items()}
    out, _ = run(**inputs)
    return out
